# revision 17
# baseline (speedup 1.0000x reference)
"""Trainium2 Bass kernel for nn_CCEncoderVarDepth (18-conv CNN encoder + BN + SiLU + FC).

Sharding: data-parallel over batch (8 samples -> 8 cores) for stages 1-5,
channel-parallel (128 of 1024 output channels per core, all samples) for
stage 6. BatchNorm statistics for stages 3-5 are combined with tiny
AllGather collectives; stage-6 BN is fully local. The final FC is computed
redundantly on every core for all 8 samples.

Layouts: channels on partitions. The 32/64-channel stages pack 4/2 spatial
row-bands of the image across partition blocks; the conv then runs as a
single dense matmul per tile with HOST-BUILT BLOCK-DIAGONAL weights (the
zero blocks cost nothing extra: the PE streams N columns regardless).
3x3 conv taps are offset views into zero-padded SBUF activations,
accumulated in PSUM (9 matmuls per output tile). Matmul operands bf16,
PSUM/statistics fp32. ScalarE only ever runs Silu (one ACT table load);
copies run on DVE; rsqrt for BN uses the bit-trick + 2 Newton steps on DVE.
"""
import sys

if "/opt/trn_rl_repo" not in sys.path:
    sys.path.insert(0, "/opt/trn_rl_repo")

import numpy as np
import ml_dtypes

import concourse.bass as bass
import concourse.mybir as mybir
from concourse import tile
from concourse.bass_utils import run_bass_kernel_spmd
from concourse.vector_clock import ScopedClock, VectorClock

dt = mybir.dt
BF = dt.bfloat16
F32 = dt.float32
U32 = dt.uint32
AF = mybir.ActivationFunctionType
ALU = mybir.AluOpType
N_CORES = 8
EPS = 1e-5
MAGIC = 0x5F3759DF

CONV_SPECS = [
    (1, 32, 2), (32, 32, 1), (32, 32, 1),
    (32, 64, 2), (64, 64, 1), (64, 64, 1),
    (64, 128, 2), (128, 128, 1), (128, 128, 1),
    (128, 256, 2), (256, 256, 1), (256, 256, 1),
    (256, 512, 2), (512, 512, 1), (512, 512, 1),
    (512, 1024, 2), (1024, 1024, 1), (1024, 1024, 1),
]
KC = {7: 1, 8: 1, 9: 1, 10: 2, 11: 2, 12: 2, 13: 4, 14: 4, 15: 4, 16: 8, 17: 8}
MC = {7: 1, 8: 1, 9: 2, 10: 2, 11: 2, 12: 4, 13: 4, 14: 4, 15: 1, 16: 1, 17: 1}
BN_MC = [1, 1, 1, 2, 2, 2, 4, 4, 4, 1, 1]
SLOT_COL = [0, 1, 2, 3, 5, 7, 9, 13, 17, 21, 22]


class ChunkedDrainTileContext(tile.TileContext):
    """The deployed walrus supports only ONE sync-wait command per
    instruction. Tile's wait assignment freely emits several. Fixes:
    (1) kernel-tail drain replaced by a chain of single-wait sync NOPs;
    (2) post-lowering, any instruction with k>1 waits has k-1 hoisted onto
    same-engine NOPs inserted right before it (engines execute in order)."""

    def _drain_and_barrier(self, tick_clock, wait_clock):
        gc = tick_clock.global_clock
        n = len(gc)
        for i in range(n):
            if gc[i] > 0:
                vc = VectorClock([0] * n)
                vc.require_at_least(i, gc[i])
                nop = self.nc.sync.nop()
                wait_clock.add_sem_waits(nop.ins, ScopedClock({None: vc}))
        self.nc.sync.drain()
        self.nc.all_engine_barrier()
        popped = self.nc._tile_sem_poison_stack.pop()
        assert popped is self._sem_poison
        self.nc.clear_and_free_semaphores(list(self.sems.allocated().values()))
        self.nc.all_engine_barrier()
        self._split_multi_waits()

    def _split_multi_waits(self):
        nc = self.nc
        for f in nc.m.functions:
            for blk in f.blocks:
                insts = blk.bb.instructions if hasattr(blk, "bb") else blk.instructions
                new = []
                for inst in insts:
                    si = inst.sync_info
                    if si is not None and si.on_wait and len(si.on_wait) > 1:
                        waits = list(si.on_wait)
                        for w in waits[:-1]:
                            nop = mybir.InstNoOp(
                                name=f"{inst.name}_sw{len(new)}", ins=[], outs=[])
                            nop.engine = inst.engine
                            nop.sync_info = mybir.SyncInfo(
                                on_wait=[w], on_update=[])
                            nc.register_instruction(nop, overwrite=True)
                            new.append(nop)
                        inst.sync_info = mybir.SyncInfo(
                            on_wait=[waits[-1]], on_update=list(si.on_update))
                    new.append(inst)
                insts[:] = new


def build_nc():
    nc = bass.Bass()
    P = nc.declare_dram_parameter

    p_im = P("im2col", [128, 12288], BF, isOutput=False)
    p_ws = {0: P("w0", [128, 128], BF, isOutput=False)}
    for i, cols in [(1, 128), (2, 128), (3, 256), (4, 128), (5, 128), (6, 256)]:
        p_ws[i] = P(f"w{i}", [9, 128, cols], BF, isOutput=False)
    for i in range(7, 18):
        p_ws[i] = P(f"w{i}", [9, 128, KC[i] * MC[i] * 128], BF, isOutput=False)
    p_bng = P("bng", [128, 23], F32, isOutput=False)
    p_bnb = P("bnb", [128, 23], F32, isOutput=False)
    p_fcw = P("fcw", [8, 128, 128], F32, isOutput=False)
    p_fcb = P("fcb", [128, 1], F32, isOutput=False)
    p_out = P("fc", [128, 8], F32, isOutput=True)

    rg = [list(range(N_CORES))]

    with ChunkedDrainTileContext(nc) as tc:
        with (
            tc.tile_pool(name="acts", bufs=1) as acts,
            tc.tile_pool(name="wsm", bufs=22) as wsm,
            tc.tile_pool(name="wdp", bufs=6) as wdp,
            tc.tile_pool(name="imp", bufs=3) as imp,
            tc.tile_pool(name="stat", bufs=2) as stat,
            tc.tile_pool(name="ps", bufs=4, space="PSUM") as psp,
            tc.tile_pool(name="dram", bufs=2, space="DRAM") as dram,
        ):
            def abuf(name, free, dtype=BF):
                return acts.tile([128, free], dtype, tag=name, name=name)

            b1a = abuf("b1a", 98 * 130)
            b1b = abuf("b1b", 98 * 130)
            b2a = abuf("b2a", 98 * 66)
            b2b = abuf("b2b", 98 * 66)
            b3a = abuf("b3a", 98 * 34)
            b3b = abuf("b3b", 98 * 34)
            raw3 = abuf("raw3", 3072)
            b4a = abuf("b4a", 2 * 50 * 18)
            b4b = abuf("b4b", 2 * 50 * 18)
            raw4 = acts.tile([128, 2 * 768], BF, tag="raw3", name="raw4")
            b5a = abuf("b5a", 4 * 26 * 10)
            b5b = abuf("b5b", 4 * 26 * 10)
            raw5 = acts.tile([128, 4 * 192], BF, tag="b3b", name="raw5")
            # stage-6 tiles reuse slots of buffers that are dead by then
            cp6i = acts.tile([128, 4 * 8 * 192], BF, tag="b1a", name="cp6i")
            b6in = abuf("b6in", 4 * 8 * 26 * 10)
            cp6y = acts.tile([128, 8 * 8 * 48], BF, tag="b1b", name="cp6y")
            b6y = acts.tile([128, 8 * 8 * 14 * 6], BF, tag="b2a", name="b6y")
            b6z = acts.tile([128, 8 * 8 * 14 * 6], BF, tag="b2b", name="b6z")
            raw6 = abuf("raw6", 384)
            cmp6 = abuf("cmp6", 384)
            t17 = acts.tile([128, 384], F32, tag="b3a", name="t17")
            cmp5 = acts.tile([128, 4 * 192], BF, tag="raw3", name="cmp5")
            pooled = abuf("pooled", 8, F32)
            bng = abuf("bng", 23, F32)
            bnb = abuf("bnb", 23, F32)
            fcb = abuf("fcb", 1, F32)
            cmagic = abuf("cmagic", 4, U32)
            c15f = abuf("c15f", 4, F32)

            nc.sync.dma_start(bng[:], p_bng[:])
            nc.sync.dma_start(bnb[:], p_bnb[:])
            nc.sync.dma_start(fcb[:], p_fcb[:])
            nc.vector.memset(cmagic[:], MAGIC)
            nc.vector.memset(c15f[:], 1.5)
            # warmup collective: absorbs first-collective setup latency while
            # conv0-conv2 run (no data dependency)
            wci = dram.tile([128, 2], F32, tag="wci", name="wci")
            wco = dram.tile([N_CORES * 128, 2], F32, tag="wco", name="wco")
            nc.sync.dma_start(wci[:], c15f[:, 0:2])
            nc.gpsimd.collective_compute(
                "AllGather", ALU.bypass, replica_groups=rg,
                ins=[wci.opt()], outs=[wco.opt()])

            def view(buf, *dims):
                pat = " ".join(f"d{i}" for i in range(len(dims)))
                kw = {f"d{i}": d for i, d in enumerate(dims)}
                return buf[:].rearrange(f"p ({pat}) -> p {pat}", **kw)

            v1a, v1b = view(b1a, 98, 130), view(b1b, 98, 130)
            v2a, v2b = view(b2a, 98, 66), view(b2b, 98, 66)
            v3a, v3b = view(b3a, 98, 34), view(b3b, 98, 34)
            v4a, v4b = view(b4a, 2, 50, 18), view(b4b, 2, 50, 18)
            v5a, v5b = view(b5a, 4, 26, 10), view(b5b, 4, 26, 10)
            v6in = view(b6in, 4, 8, 26, 10)
            v6y, v6z = view(b6y, 8, 8, 14, 6), view(b6z, 8, 8, 14, 6)

            for v in (v1a, v1b, v2a, v2b, v3a, v3b):
                nc.vector.memset(v[:, :, 0], 0.0)
                nc.vector.memset(v[:, :, v.shape[2] - 1], 0.0)
                nc.vector.memset(v[:, 0, :], 0.0)
                nc.vector.memset(v[:, v.shape[1] - 1, :], 0.0)
            for v in (v4a, v4b, v5a, v5b):
                nc.vector.memset(v[:, :, :, 0], 0.0)
                nc.vector.memset(v[:, :, :, v.shape[3] - 1], 0.0)
                nc.vector.memset(v[:, :, 0, :], 0.0)
                nc.vector.memset(v[:, :, v.shape[2] - 1, :], 0.0)
            for v in (v6in, v6y, v6z):
                nc.vector.memset(v[:, :, :, :, 0], 0.0)
                nc.vector.memset(v[:, :, :, :, v.shape[4] - 1], 0.0)
                nc.vector.memset(v[:, :, :, 0, :], 0.0)
                nc.vector.memset(v[:, :, :, v.shape[3] - 1, :], 0.0)

            def load_w_taps(i, cols):
                ts = []
                for t in range(9):
                    w = wsm.tile([128, cols], BF, tag="w", name=f"w{i}_{t}")
                    nc.sync.dma_start(w[:], p_ws[i][t])
                    ts.append(w)
                return ts

            def halos(buf, bw, pc, rows):
                # next blocks' top halo <- prev blocks' last interior row
                nc.sync.dma_start(
                    buf[bw:128, 0:pc],
                    buf[0:128 - bw, rows * pc:(rows + 1) * pc])
                # prev blocks' bottom halo <- next blocks' first interior row
                nc.sync.dma_start(
                    buf[0:128 - bw, (rows + 1) * pc:(rows + 2) * pc],
                    buf[bw:128, pc:2 * pc])

            # =========== conv0: host im2col, K=9 blockdiag ===========
            w0t = wsm.tile([128, 128], BF, tag="w", name="w0t")
            nc.sync.dma_start(w0t[:], p_ws[0][:])
            for it in range(12):
                imt = imp.tile([128, 1024], BF, name="imt")
                nc.sync.dma_start(imt[:], p_im[:, 1024 * it:1024 * (it + 1)])
                ps = psp.tile([128, 1024], F32, tag="ps", name="ps0")
                for g in range(2):
                    nc.tensor.matmul(ps[:, 512 * g:512 * g + 512], w0t[:],
                                     imt[:, 512 * g:512 * g + 512],
                                     start=True, stop=True)
                nc.scalar.activation(
                    v1a[:, 1 + 8 * it:9 + 8 * it, 1:129], ps[:], AF.Silu)
            halos(b1a, 32, 130, 96)

            # ====== conv1 / conv2: 32ch stride-1, 4-block blockdiag ======
            def conv_s1(src_v, dst_v, dst_buf, wt):
                for it in range(12):
                    ps = psp.tile([128, 1024], F32, tag="ps", name="pss1")
                    for g in range(2):
                        r0 = 8 * it + 4 * g
                        for t in range(9):
                            dy, dx = t // 3, t % 3
                            rhs = src_v[:, r0 + dy:r0 + dy + 4, dx:dx + 128]
                            nc.tensor.matmul(
                                ps[:, 512 * g:512 * g + 512], wt[t][:], rhs,
                                start=(t == 0), stop=(t == 8))
                    nc.scalar.activation(
                        dst_v[:, 1 + 8 * it:9 + 8 * it, 1:129], ps[:], AF.Silu)
                halos(dst_buf, 32, 130, 96)

            conv_s1(v1a, v1b, b1b, load_w_taps(1, 128))
            conv_s1(v1b, v1a, b1a, load_w_taps(2, 128))

            # ====== conv3: 32->64 stride-2, two b-variant matmuls ======
            w3t = load_w_taps(3, 256)
            for it in range(6):
                ps = psp.tile([128, 1024], F32, tag="ps", name="ps3")
                for b in range(2):
                    r0 = 8 * it
                    for t in range(9):
                        dy, dx = t // 3, t % 3
                        rhs = v1a[:, 2 * r0 + dy:2 * r0 + dy + 16:2,
                                        dx:dx + 128:2]
                        nc.tensor.matmul(
                            ps[:, 512 * b:512 * b + 512],
                            w3t[t][:, 128 * b:128 * b + 128], rhs,
                            start=(t == 0), stop=(t == 8))
                for b in range(2):
                    nc.scalar.activation(
                        v2a[:, 1 + 48 * b + 8 * it:9 + 48 * b + 8 * it, 1:65],
                        ps[:, 512 * b:512 * b + 512], AF.Silu)
            halos(b2a, 64, 66, 96)

            # ====== conv4 / conv5: 64ch stride-1, 2-block blockdiag ======
            def conv_s2(src_v, dst_v, dst_buf, wt):
                for it in range(6):
                    ps = psp.tile([128, 1024], F32, tag="ps", name="pss2")
                    for g in range(2):
                        r0 = 16 * it + 8 * g
                        for t in range(9):
                            dy, dx = t // 3, t % 3
                            rhs = src_v[:, r0 + dy:r0 + dy + 8, dx:dx + 64]
                            nc.tensor.matmul(
                                ps[:, 512 * g:512 * g + 512], wt[t][:], rhs,
                                start=(t == 0), stop=(t == 8))
                    nc.scalar.activation(
                        dst_v[:, 1 + 16 * it:17 + 16 * it, 1:65], ps[:], AF.Silu)
                halos(dst_buf, 64, 66, 96)

            conv_s2(v2a, v2b, b2b, load_w_taps(4, 128))
            conv_s2(v2b, v2a, b2a, load_w_taps(5, 128))

            # ---- split-half BN machinery ----
            def bn_stats_part(raw, ranges, mc, nm):
                """ranges: list per m of (start, length) in raw. -> (st2, AG out)"""
                st2 = stat.tile([128, 2 * mc], F32, tag="st2", name="st2" + nm)
                for m, (st, ln) in enumerate(ranges):
                    gdim = (ln + 511) // 512
                    fsz = ln // gdim
                    assert ln % gdim == 0
                    bns = stat.tile([128, 6 * gdim], F32, tag="bns",
                                    name="bns" + nm)
                    for gi in range(gdim):
                        nc.vector.bn_stats(
                            bns[:, 6 * gi:6 * gi + 6],
                            raw[:, st + gi * fsz:st + (gi + 1) * fsz])
                    nc.vector.bn_aggr(st2[:, 2 * m:2 * m + 2],
                                      bns[:].rearrange("p (g f) -> p g f", f=6))
                ccin = dram.tile([128, 2 * mc], F32, tag="ccin", name="cci" + nm)
                ccout = dram.tile([N_CORES * 128, 2 * mc], F32, tag="ccout",
                                  name="cco" + nm)
                nc.sync.dma_start(ccin[:], st2[:])
                nc.gpsimd.collective_compute(
                    "AllGather", ALU.bypass, replica_groups=rg,
                    ins=[ccin.opt()], outs=[ccout.opt()])
                gath = stat.tile([128, 8 * 2 * mc], F32, tag="gath",
                                 name="gth" + nm)
                nc.sync.dma_start(
                    gath[:], ccout[:].rearrange("(r p) m -> p r m", p=128))
                return gath

            def bn_apply(gaths_w, raw, px, mc, slot, dst_views):
                """gaths_w: list of (gath tile, weight). Combine E[x], E[x^2]
                across ranks and halves, then normalize+SiLU."""
                mg = stat.tile([128, mc], F32, tag="mg", name="mg")
                vg = stat.tile([128, mc], F32, tag="vg", name="vg")
                sc1 = stat.tile([128, mc], F32, tag="sc1", name="sc1")
                sc2 = stat.tile([128, mc], F32, tag="sc2", name="sc2")
                t2 = stat.tile([128, mc], F32, tag="t2", name="t2")
                ab = stat.tile([128, 2 * mc], F32, tag="ab", name="ab")
                a, bvec = ab[:, 0:mc], ab[:, mc:2 * mc]
                tmp = stat.tile([128, mc * 8], F32, tag="tmp", name="tmp")
                tv = tmp[:].rearrange("p (m s) -> p m s", m=mc)
                first = True
                for gath, wgt in gaths_w:
                    gv = gath[:].rearrange("p (s m) -> p s m", s=8)
                    means = gv[:, :, 0:2 * mc:2].rearrange("p s m -> p m s")
                    varis = gv[:, :, 1:2 * mc:2].rearrange("p s m -> p m s")
                    nc.vector.reduce_sum(sc1[:], means, axis=mybir.AxisListType.X)
                    nc.vector.tensor_mul(tv, means, means)
                    nc.vector.tensor_add(tv, tv, varis)
                    nc.vector.reduce_sum(sc2[:], tv, axis=mybir.AxisListType.X)
                    if first:
                        nc.vector.tensor_scalar_mul(mg[:], sc1[:], wgt * 0.125)
                        nc.vector.tensor_scalar_mul(vg[:], sc2[:], wgt * 0.125)
                        first = False
                    else:
                        nc.vector.tensor_scalar(sc1[:], sc1[:], wgt * 0.125,
                                                None, op0=ALU.mult)
                        nc.vector.tensor_add(mg[:], mg[:], sc1[:])
                        nc.vector.tensor_scalar(sc2[:], sc2[:], wgt * 0.125,
                                                None, op0=ALU.mult)
                        nc.vector.tensor_add(vg[:], vg[:], sc2[:])
                nc.vector.tensor_mul(sc1[:], mg[:], mg[:])
                nc.vector.tensor_sub(vg[:], vg[:], sc1[:])
                nc.vector.tensor_scalar_add(vg[:], vg[:], EPS)
                vgi = vg[:].bitcast(U32)
                yi = sc1[:].bitcast(U32)
                nc.vector.tensor_scalar(yi, vgi, 1, None,
                                        op0=ALU.logical_shift_right)
                nc.vector.tensor_sub(yi, cmagic[:, 0:mc], yi)
                y = sc1[:]
                h = sc2[:]
                nc.vector.tensor_scalar_mul(h, vg[:], 0.5)
                for _ in range(2):
                    nc.vector.tensor_mul(t2[:], y, y)
                    nc.vector.tensor_mul(t2[:], t2[:], h)
                    nc.vector.tensor_sub(t2[:], c15f[:, 0:mc], t2[:])
                    nc.vector.tensor_mul(y, y, t2[:])
                col = SLOT_COL[slot]
                nc.vector.tensor_mul(a, y, bng[:, col:col + mc])
                nc.vector.tensor_mul(mg[:], mg[:], a)
                nc.vector.tensor_sub(bvec, bnb[:, col:col + mc], mg[:])
                for m in range(mc):
                    nc.scalar.activation(
                        dst_views[m], raw[:, m * px:(m + 1) * px], AF.Silu,
                        bias=ab[:, mc + m:mc + m + 1], scale=ab[:, m:m + 1])

            # ---- BN stats + AllGather + normalize+SiLU (batch-DP) ----
            def bn_silu(raw, px, mc, slot, dst_views):
                g = bn_stats_part(raw, [(m * px, px) for m in range(mc)],
                                  mc, f"f{slot}")
                bn_apply([(g, 1.0)], raw, px, mc, slot, dst_views)

            def bn_silu_local(raw, slot, dst_view):
                bns = stat.tile([128, 6], F32, tag="bns", name="bnsl")
                nc.vector.bn_stats(bns[:], raw[:])
                st2 = stat.tile([128, 2], F32, tag="st2", name="st2l")
                nc.vector.bn_aggr(st2[:], bns[:].rearrange("p (g f) -> p g f", f=6))
                ab = stat.tile([128, 2], F32, tag="ab", name="abl")
                vg = stat.tile([128, 1], F32, tag="vg", name="vgl")
                sc2 = stat.tile([128, 1], F32, tag="sc2", name="sc2l")
                t2 = stat.tile([128, 1], F32, tag="t2", name="t2l")
                nc.vector.tensor_scalar_add(vg[:], st2[:, 1:2], EPS)
                y = ab[:, 0:1]
                yi = y.bitcast(U32)
                nc.vector.tensor_scalar(yi, vg[:].bitcast(U32), 1, None,
                                        op0=ALU.logical_shift_right)
                nc.vector.tensor_sub(yi, cmagic[:, 0:1], yi)
                nc.vector.tensor_scalar_mul(sc2[:], vg[:], 0.5)
                for _ in range(2):
                    nc.vector.tensor_mul(t2[:], y, y)
                    nc.vector.tensor_mul(t2[:], t2[:], sc2[:])
                    nc.vector.tensor_sub(t2[:], c15f[:, 0:1], t2[:])
                    nc.vector.tensor_mul(y, y, t2[:])
                col = SLOT_COL[slot]
                nc.vector.tensor_mul(y, y, bng[:, col:col + 1])
                nc.vector.tensor_mul(vg[:], st2[:, 0:1], y)
                nc.vector.tensor_sub(ab[:, 1:2], bnb[:, col:col + 1], vg[:])
                nc.scalar.activation(dst_view, raw[:], AF.Silu,
                                     bias=ab[:, 1:2], scale=ab[:, 0:1])

            # ====== conv6: 64->128 stride-2, two b-variants -> raw3 ======
            w6t = load_w_taps(6, 256)
            g6 = []
            for it in range(3):
                ps = psp.tile([128, 1024], F32, tag="ps", name="ps6")
                for b in range(2):
                    r0 = 16 * it
                    for t in range(9):
                        dy, dx = t // 3, t % 3
                        rhs = v2a[:, 2 * r0 + dy:2 * r0 + dy + 32:2,
                                  dx:dx + 64:2]
                        nc.tensor.matmul(
                            ps[:, 512 * b:512 * b + 512],
                            w6t[t][:, 128 * b:128 * b + 128], rhs,
                            start=(t == 0), stop=(t == 8))
                for b in range(2):
                    nc.vector.tensor_copy(
                        raw3[:, (48 * b + 16 * it) * 32:(48 * b + 16 * it + 16) * 32],
                        ps[:, 512 * b:512 * b + 512])
                if it == 1:
                    # px 0..1023 (rows 0..31) complete after its 0-1
                    g6.append(bn_stats_part(raw3[:], [(0, 1024)], 1, "c6a"))
            g6.append(bn_stats_part(raw3[:], [(1024, 2048)], 1, "c6b"))
            bn_apply([(g6[0], 1.0 / 3), (g6[1], 2.0 / 3)],
                     raw3[:], 3072, 1, 0, [v3a[:, 1:97, 1:33]])

            # per-tap deep weight tile: [128, kc*mc*128], one DMA per tap
            def wtap(i, t):
                w = wdp.tile([128, KC[i] * MC[i] * 128], BF, tag="wd",
                             name=f"w{i}_{t}")
                nc.sync.dma_start(w[:], p_ws[i][t])
                return w

            # ====== conv7 / conv8: 128ch ======
            def conv_s3(i, src_v, slot, dst_views):
                wt = []
                for t in range(9):
                    w = wsm.tile([128, 128], BF, tag="w", name=f"w{i}_{t}")
                    nc.sync.dma_start(w[:], p_ws[i][t])
                    wt.append(w)
                gs = []
                for g in range(3):
                    ps = psp.tile([128, 1024], F32, tag="ps", bufs=4,
                                  name=f"ps{i}_{g}")
                    for t in range(9):
                        dy, dx = t // 3, t % 3
                        for h in range(2):
                            r0 = 32 * g + 16 * h
                            rhs = src_v[:, r0 + dy:r0 + dy + 16, dx:dx + 32]
                            nc.tensor.matmul(
                                ps[:, 512 * h:512 * h + 512], wt[t][:], rhs,
                                start=(t == 0), stop=(t == 8))
                    nc.vector.tensor_copy(
                        raw3[:, 1024 * g:1024 * (g + 1)], ps[:])
                    if g == 1:
                        gs.append(bn_stats_part(raw3[:], [(0, 2048)], 1,
                                                f"c{i}a"))
                gs.append(bn_stats_part(raw3[:], [(2048, 1024)], 1, f"c{i}b"))
                bn_apply([(gs[0], 2.0 / 3), (gs[1], 1.0 / 3)],
                         raw3[:], 3072, 1, slot, dst_views)

            conv_s3(7, v3a, 1, [v3b[:, 1:97, 1:33]])
            conv_s3(8, v3b, 2, [v3a[:, 1:97, 1:33]])

            # ====== conv9: 128->256 stride-2 ======
            wt9 = []
            for t in range(9):
                w = wsm.tile([128, 256], BF, tag="w", name=f"w9_{t}")
                nc.sync.dma_start(w[:], p_ws[9][t])
                wt9.append(w)
            g9 = []
            for cch in range(2):
                for m in range(2):
                    ps = psp.tile([128, 384], F32, tag="ps", bufs=4,
                                  name=f"ps9_{m}_{cch}")
                    r0 = 24 * cch
                    for t in range(9):
                        dy, dx = t // 3, t % 3
                        rhs = v3a[:, 2 * r0 + dy:2 * r0 + dy + 48:2,
                                  dx:dx + 32:2]
                        nc.tensor.matmul(
                            ps[:], wt9[t][:, 128 * m:128 * m + 128],
                            rhs, start=(t == 0), stop=(t == 8))
                    nc.vector.tensor_copy(
                        raw4[:, m * 768 + 384 * cch:m * 768 + 384 * (cch + 1)],
                        ps[:])
                if cch == 0:
                    g9.append(bn_stats_part(
                        raw4[:], [(0, 384), (768, 384)], 2, "c9a"))
            g9.append(bn_stats_part(raw4[:], [(384, 384), (1152, 384)], 2, "c9b"))
            bn_apply([(g9[0], 0.5), (g9[1], 0.5)],
                     raw4[:], 768, 2, 3, [v4a[:, m, 1:49, 1:17] for m in range(2)])

            # ====== conv10 / conv11: 256ch ======
            def conv_s4(i, src_v, slot, dst_views):
                wt = []
                for t in range(9):
                    w = wsm.tile([128, 512], BF, tag="w", name=f"w{i}_{t}")
                    nc.sync.dma_start(w[:], p_ws[i][t])
                    wt.append(w)
                gs = []
                for cch in range(2):
                    for m in range(2):
                        ps = psp.tile([128, 384], F32, tag="ps", bufs=4,
                                      name=f"ps{i}_{m}_{cch}")
                        r0 = 24 * cch
                        acc = 0
                        for t in range(9):
                            dy, dx = t // 3, t % 3
                            for k in range(2):
                                acc += 1
                                rhs = src_v[:, k, r0 + dy:r0 + dy + 24,
                                            dx:dx + 16]
                                nc.tensor.matmul(
                                    ps[:],
                                    wt[t][:, (2 * k + m) * 128:(2 * k + m) * 128 + 128],
                                    rhs, start=(acc == 1), stop=(acc == 18))
                        nc.vector.tensor_copy(
                            raw4[:, m * 768 + 384 * cch:m * 768 + 384 * (cch + 1)],
                            ps[:])
                    if cch == 0:
                        gs.append(bn_stats_part(
                            raw4[:], [(0, 384), (768, 384)], 2, f"c{i}a"))
                gs.append(bn_stats_part(raw4[:], [(384, 384), (1152, 384)],
                                        2, f"c{i}b"))
                bn_apply([(gs[0], 0.5), (gs[1], 0.5)],
                         raw4[:], 768, 2, slot, dst_views)

            conv_s4(10, v4a, 4, [v4b[:, m, 1:49, 1:17] for m in range(2)])
            conv_s4(11, v4b, 5, [v4a[:, m, 1:49, 1:17] for m in range(2)])

            # ====== conv12: 256->512 stride-2 ======
            pss = [psp.tile([128, 192], F32, tag="ps", bufs=4,
                            name=f"ps12_{j}") for j in range(4)]
            acc = 0
            for t in range(9):
                dy, dx = t // 3, t % 3
                w = wtap(12, t)
                for k in range(2):
                    acc += 1
                    for m in range(4):
                        rhs = v4a[:, k, dy:dy + 48:2, dx:dx + 16:2]
                        nc.tensor.matmul(
                            pss[m][:],
                            w[:, (4 * k + m) * 128:(4 * k + m) * 128 + 128],
                            rhs, start=(acc == 1), stop=(acc == 18))
            for m in range(4):
                nc.vector.tensor_copy(raw5[:, m * 192:(m + 1) * 192], pss[m][:])
            bn_silu(raw5[:], 192, 4, 6, [v5a[:, m, 1:25, 1:9] for m in range(4)])

            # ====== conv13 / conv14: 512ch ======
            def conv_s5(i, src_v, slot, dst_views):
                pss = [psp.tile([128, 192], F32, tag="ps", bufs=4,
                                name=f"ps{i}_{j}") for j in range(4)]
                acc = 0
                for t in range(9):
                    dy, dx = t // 3, t % 3
                    w = wtap(i, t)
                    for k in range(4):
                        acc += 1
                        for m in range(4):
                            rhs = src_v[:, k, dy:dy + 24, dx:dx + 8]
                            nc.tensor.matmul(
                                pss[m][:],
                                w[:, (4 * k + m) * 128:(4 * k + m) * 128 + 128],
                                rhs, start=(acc == 1), stop=(acc == 36))
                for m in range(4):
                    nc.vector.tensor_copy(raw5[:, m * 192:(m + 1) * 192],
                                          pss[m][:])
                bn_silu(raw5[:], 192, 4, slot, dst_views)

            conv_s5(13, v5a, 7, [v5b[:, m, 1:25, 1:9] for m in range(4)])
            conv_s5(14, v5b, 8, [cmp5[:, m * 192:(m + 1) * 192] for m in range(4)])

            # ====== stage 6: channel-parallel, AllGather activations ======
            cc6i = dram.tile([128, 4 * 192], BF, tag="cc6i", name="cc6i")
            cc6o = dram.tile([N_CORES * 128, 4 * 192], BF, tag="cc6o",
                             name="cc6o")
            nc.sync.dma_start(cc6i[:], cmp5[:])
            nc.gpsimd.collective_compute(
                "AllGather", ALU.bypass, replica_groups=rg,
                ins=[cc6i.opt()], outs=[cc6o.opt()])
            cc6v = cc6o[:].rearrange("(r p) px -> p r px", p=128)
            nc.sync.dma_start(cp6i[:].rearrange("p (r px) -> p r px", r=8),
                              cc6v)
            # scatter compact [p, s, k, a, b] -> padded (DVE, strided APs)
            cp6iv = cp6i[:].rearrange("p (s k a b) -> p s k a b",
                                      s=8, k=4, a=24)
            for k in range(4):
                nc.vector.tensor_copy(
                    v6in[:, k, :, 1:25, 1:9],
                    cp6iv[:, :, k].rearrange("p s a b -> p s a b"))

            def stage6_conv(i, rhs_of, dst_raw, copy_on_dve=True):
                ps = psp.tile([128, 384], F32, tag="ps", bufs=4,
                              name=f"ps{i}")
                acc = 0
                nk = KC[i]
                for t in range(9):
                    dy, dx = t // 3, t % 3
                    w = wtap(i, t)
                    for k in range(nk):
                        acc += 1
                        nc.tensor.matmul(
                            ps[:], w[:, 128 * k:128 * k + 128],
                            rhs_of(k, dy, dx),
                            start=(acc == 1), stop=(acc == 9 * nk))
                nc.vector.tensor_copy(dst_raw[:], ps[:])

            def ag_to(dst_v, src_cmp, tag):
                ci = dram.tile([128, 384], BF, tag=tag + "i", name=tag + "i")
                co = dram.tile([N_CORES * 128, 384], BF, tag=tag + "o",
                               name=tag + "o")
                nc.sync.dma_start(ci[:], src_cmp[:])
                nc.gpsimd.collective_compute(
                    "AllGather", ALU.bypass, replica_groups=rg,
                    ins=[ci.opt()], outs=[co.opt()])
                cov = co[:].rearrange("(k p) px -> p k px", p=128)
                nc.sync.dma_start(
                    cp6y[:].rearrange("p (k px) -> p k px", k=8), cov)
                cpv = cp6y[:].rearrange("p (k s a b) -> p k s a b",
                                        k=8, s=8, a=12)
                for k in range(8):
                    nc.vector.tensor_copy(dst_v[:, k, :, 1:13, 1:5], cpv[:, k])

            stage6_conv(15, lambda k, dy, dx:
                        v6in[:, k, :, dy:dy + 24:2, dx:dx + 8:2], raw6)
            bn_silu_local(raw6, 9,
                          cmp6[:].rearrange("p (s a b) -> p s a b", s=8, a=12))
            ag_to(v6y, cmp6, "ag15")

            stage6_conv(16, lambda k, dy, dx:
                        v6y[:, k, :, dy:dy + 12, dx:dx + 4], raw6)
            bn_silu_local(raw6, 10,
                          cmp6[:].rearrange("p (s a b) -> p s a b", s=8, a=12))
            ag_to(v6z, cmp6, "ag16")

            stage6_conv(17, lambda k, dy, dx:
                        v6z[:, k, :, dy:dy + 12, dx:dx + 4], t17)
            nc.vector.reduce_sum(
                pooled[:], t17[:].rearrange("p (s f) -> p s f", s=8),
                axis=mybir.AxisListType.X)

            # ====== FC (fp32 matmul, all samples, redundant per core) ======
            fci = dram.tile([128, 8], F32, tag="fci", name="fci")
            fco = dram.tile([N_CORES * 128, 8], F32, tag="fco", name="fco")
            nc.sync.dma_start(fci[:], pooled[:])
            nc.gpsimd.collective_compute(
                "AllGather", ALU.bypass, replica_groups=rg,
                ins=[fci.opt()], outs=[fco.opt()])
            fcov = fco[:].rearrange("(k p) s -> p k s", p=128)
            psf = psp.tile([128, 8], F32, tag="ps", bufs=4, name="psf")
            frs = stat.tile([128, 64], F32, tag="fr", name="frs")
            nc.sync.dma_start(frs[:].rearrange("p (k s) -> p k s", k=8), fcov)
            for k in range(8):
                fw = wdp.tile([128, 128], F32, tag="wd", name=f"fw{k}")
                nc.sync.dma_start(fw[:], p_fcw[k])
                nc.tensor.matmul(psf[:], fw[:], frs[:, 8 * k:8 * k + 8],
                                 start=(k == 0), stop=(k == 7))
            fout = stat.tile([128, 8], F32, tag="fout", name="fout")
            nc.vector.tensor_scalar_add(fout[:], psf[:], fcb[:, 0:1])
            nc.sync.dma_start(p_out[:], fout[:])
    return nc


# ====================== host side ======================

_CACHED_NC = None


def _get_nc():
    global _CACHED_NC
    if _CACHED_NC is None:
        _CACHED_NC = build_nc()
    return _CACHED_NC


def _to_bf(a):
    return np.asarray(a, np.float32).astype(ml_dtypes.bfloat16)


def _im2col0(xs):
    """xs: [768, 256]. -> [128, 12288] bf16: partitions 32q+t hold tap t of
    quarter q (stride-2, pad 1), 96 rows x 128 cols each."""
    xp = np.zeros((770, 258), np.float32)
    xp[1:769, 1:257] = xs
    out = np.zeros((128, 12288), np.float32)
    for q in range(4):
        for t in range(9):
            dy, dx = t // 3, t % 3
            rows = xp[192 * q + dy:192 * q + dy + 192:2, dx:dx + 256:2]
            out[32 * q + t] = rows.reshape(-1)
    return _to_bf(out)


def _pack_weights(conv_ws):
    w = [np.asarray(x, np.float32) for x in conv_ws]
    d = {}
    # conv0 blockdiag [128, 128]: rows 32q+t -> cols 32q..32q+31
    a = np.zeros((128, 128), np.float32)
    for q in range(4):
        for t in range(9):
            a[32 * q + t, 32 * q:32 * q + 32] = w[0][t // 3, t % 3, 0, :]
    d["w0"] = _to_bf(a)
    # conv1/2: [9, 128, 128] 4-block diagonal
    for i in (1, 2):
        a = np.zeros((9, 128, 128), np.float32)
        for t in range(9):
            blk = w[i][t // 3, t % 3]          # [32, 32]
            for q in range(4):
                a[t, 32 * q:32 * q + 32, 32 * q:32 * q + 32] = blk
        d[f"w{i}"] = _to_bf(a)
    # conv3: [9, 128, 256]; variant b at cols 128b: rows 32(2h+b) -> 64h
    a = np.zeros((9, 128, 256), np.float32)
    for t in range(9):
        blk = w[3][t // 3, t % 3]              # [32, 64]
        for b in range(2):
            for h in range(2):
                qb = 2 * h + b
                a[t, 32 * qb:32 * qb + 32, 128 * b + 64 * h:128 * b + 64 * h + 64] = blk
    d["w3"] = _to_bf(a)
    # conv4/5: [9, 128, 128] 2-block diagonal of [64, 64]
    for i in (4, 5):
        a = np.zeros((9, 128, 128), np.float32)
        for t in range(9):
            blk = w[i][t // 3, t % 3]          # [64, 64]
            for h in range(2):
                a[t, 64 * h:64 * h + 64, 64 * h:64 * h + 64] = blk
        d[f"w{i}"] = _to_bf(a)
    # conv6: [9, 128, 256]; variant b at cols 128b: rows 64b -> all 128
    a = np.zeros((9, 128, 256), np.float32)
    for t in range(9):
        blk = w[6][t // 3, t % 3]              # [64, 128]
        for b in range(2):
            a[t, 64 * b:64 * b + 64, 128 * b:128 * b + 128] = blk
    d["w6"] = _to_bf(a)
    # conv7..14: [9, 128, kc*mc*128]; free index = (k*mc + m)*128 + co
    for i in range(7, 15):
        cin, cout, _ = CONV_SPECS[i]
        kc, mc = KC[i], MC[i]
        a = np.zeros((9, 128, kc * mc * 128), np.float32)
        for t in range(9):
            for k in range(kc):
                a[t, :, k * mc * 128:(k + 1) * mc * 128] = \
                    w[i][t // 3, t % 3, 128 * k:128 * k + 128, :]
        d[f"w{i}"] = _to_bf(a)
    # conv15..17: per-core cout slice; [9, 128, kc*128]
    for i in range(15, 18):
        kc = KC[i]
        percore = []
        for c in range(N_CORES):
            a = np.zeros((9, 128, kc * 128), np.float32)
            for t in range(9):
                for k in range(kc):
                    a[t, :, 128 * k:128 * k + 128] = \
                        w[i][t // 3, t % 3, 128 * k:128 * k + 128,
                             128 * c:128 * c + 128]
            percore.append(_to_bf(a))
        d[f"w{i}"] = percore
    return d


def _pack_bn(bn_gammas, bn_betas):
    gs = [np.asarray(g, np.float32) for g in bn_gammas]
    bs = [np.asarray(b, np.float32) for b in bn_betas]
    outg = [np.zeros((128, 23), np.float32) for _ in range(N_CORES)]
    outb = [np.zeros((128, 23), np.float32) for _ in range(N_CORES)]
    for slot in range(11):
        col, mc = SLOT_COL[slot], BN_MC[slot]
        for c in range(N_CORES):
            for m in range(mc):
                if slot >= 9:
                    sl = slice(128 * c, 128 * c + 128)
                else:
                    sl = slice(128 * m, 128 * m + 128)
                outg[c][:, col + m] = gs[slot][sl]
                outb[c][:, col + m] = bs[slot][sl]
    return outg, outb


def _run(inputs, trace=False):
    x = np.asarray(inputs["x"], np.float32)
    wd = _pack_weights(inputs["conv_ws"])
    bg, bb = _pack_bn(inputs["bn_gammas"], inputs["bn_betas"])
    fc_w = np.asarray(inputs["fc_w"], np.float32)
    fc_b = np.asarray(inputs["fc_b"], np.float32)
    fcw = np.zeros((8, 128, 128), np.float32)
    for k in range(8):
        fcw[k] = fc_w[:, 128 * k:128 * k + 128].T / 48.0
    fcb = fc_b.reshape(128, 1).astype(np.float32)

    in_maps = []
    for c in range(N_CORES):
        m = {"im2col": _im2col0(x[c, 0]), "fcw": fcw, "fcb": fcb,
             "bng": bg[c], "bnb": bb[c]}
        for i in range(18):
            key = f"w{i}"
            m[key] = wd[key][c] if i >= 15 else wd[key]
        in_maps.append(m)

    nc = _get_nc()
    res = run_bass_kernel_spmd(nc, in_maps, list(range(N_CORES)), trace=trace)
    out = np.ascontiguousarray(res.results[0]["fc"].T.astype(np.float32))
    return out, res


def kernel(**inputs):
    out, _ = _run(inputs, trace=False)
    return out


# revision 18
# speedup vs baseline: 1.1108x; 1.1108x over previous
"""Trainium2 Bass kernel for nn_CCEncoderVarDepth (18-conv CNN encoder + BN + SiLU + FC).

Sharding: data-parallel over batch (8 samples -> 8 cores) for stages 1-5,
channel-parallel (128 of 1024 output channels per core, all samples) for
stage 6. BatchNorm statistics for stages 3-5 are combined with tiny
AllGather collectives; stage-6 BN is fully local. The final FC is computed
redundantly on every core for all 8 samples.

Layouts: channels on partitions. The 32/64-channel stages pack 4/2 spatial
row-bands of the image across partition blocks; the conv then runs as a
single dense matmul per tile with HOST-BUILT BLOCK-DIAGONAL weights (the
zero blocks cost nothing extra: the PE streams N columns regardless).
3x3 conv taps are offset views into zero-padded SBUF activations,
accumulated in PSUM (9 matmuls per output tile). Matmul operands bf16,
PSUM/statistics fp32. ScalarE only ever runs Silu (one ACT table load);
copies run on DVE; rsqrt for BN uses the bit-trick + 2 Newton steps on DVE.
"""
import sys

if "/opt/trn_rl_repo" not in sys.path:
    sys.path.insert(0, "/opt/trn_rl_repo")

import numpy as np
import ml_dtypes

import concourse.bass as bass
import concourse.mybir as mybir
from concourse import tile
from concourse.bass_utils import run_bass_kernel_spmd
from concourse.vector_clock import ScopedClock, VectorClock

dt = mybir.dt
BF = dt.bfloat16
F32 = dt.float32
U32 = dt.uint32
AF = mybir.ActivationFunctionType
ALU = mybir.AluOpType
N_CORES = 8
EPS = 1e-5
MAGIC = 0x5F3759DF

CONV_SPECS = [
    (1, 32, 2), (32, 32, 1), (32, 32, 1),
    (32, 64, 2), (64, 64, 1), (64, 64, 1),
    (64, 128, 2), (128, 128, 1), (128, 128, 1),
    (128, 256, 2), (256, 256, 1), (256, 256, 1),
    (256, 512, 2), (512, 512, 1), (512, 512, 1),
    (512, 1024, 2), (1024, 1024, 1), (1024, 1024, 1),
]
KC = {7: 1, 8: 1, 9: 1, 10: 2, 11: 2, 12: 2, 13: 4, 14: 4, 15: 4, 16: 8, 17: 8}
MC = {7: 1, 8: 1, 9: 2, 10: 2, 11: 2, 12: 4, 13: 4, 14: 4, 15: 1, 16: 1, 17: 1}
BN_MC = [1, 1, 1, 2, 2, 2, 4, 4, 4, 1, 1]
SLOT_COL = [0, 1, 2, 3, 5, 7, 9, 13, 17, 21, 22]


class ChunkedDrainTileContext(tile.TileContext):
    """The deployed walrus supports only ONE sync-wait command per
    instruction. Tile's wait assignment freely emits several. Fixes:
    (1) kernel-tail drain replaced by a chain of single-wait sync NOPs;
    (2) post-lowering, any instruction with k>1 waits has k-1 hoisted onto
    same-engine NOPs inserted right before it (engines execute in order)."""

    def _drain_and_barrier(self, tick_clock, wait_clock):
        gc = tick_clock.global_clock
        n = len(gc)
        for i in range(n):
            if gc[i] > 0:
                vc = VectorClock([0] * n)
                vc.require_at_least(i, gc[i])
                nop = self.nc.sync.nop()
                wait_clock.add_sem_waits(nop.ins, ScopedClock({None: vc}))
        self.nc.sync.drain()
        self.nc.all_engine_barrier()
        popped = self.nc._tile_sem_poison_stack.pop()
        assert popped is self._sem_poison
        self.nc.clear_and_free_semaphores(list(self.sems.allocated().values()))
        self.nc.all_engine_barrier()
        self._split_multi_waits()

    def _split_multi_waits(self):
        nc = self.nc
        for f in nc.m.functions:
            for blk in f.blocks:
                insts = blk.bb.instructions if hasattr(blk, "bb") else blk.instructions
                new = []
                for inst in insts:
                    si = inst.sync_info
                    if si is not None and si.on_wait and len(si.on_wait) > 1:
                        waits = list(si.on_wait)
                        for w in waits[:-1]:
                            nop = mybir.InstNoOp(
                                name=f"{inst.name}_sw{len(new)}", ins=[], outs=[])
                            nop.engine = inst.engine
                            nop.sync_info = mybir.SyncInfo(
                                on_wait=[w], on_update=[])
                            nc.register_instruction(nop, overwrite=True)
                            new.append(nop)
                        inst.sync_info = mybir.SyncInfo(
                            on_wait=[waits[-1]], on_update=list(si.on_update))
                    new.append(inst)
                insts[:] = new


def build_nc():
    nc = bass.Bass()
    P = nc.declare_dram_parameter

    p_im = P("im2col", [128, 12288], BF, isOutput=False)
    p_ws = {0: P("w0", [128, 128], BF, isOutput=False)}
    for i, cols in [(1, 128), (2, 128), (3, 256), (4, 128), (5, 128), (6, 256)]:
        p_ws[i] = P(f"w{i}", [9, 128, cols], BF, isOutput=False)
    for i in range(7, 18):
        p_ws[i] = P(f"w{i}", [9, 128, KC[i] * MC[i] * 128], BF, isOutput=False)
    p_bng = P("bng", [128, 23], F32, isOutput=False)
    p_bnb = P("bnb", [128, 23], F32, isOutput=False)
    p_fcw = P("fcw", [8, 128, 128], F32, isOutput=False)
    p_fcb = P("fcb", [128, 1], F32, isOutput=False)
    p_out = P("fc", [128, 8], F32, isOutput=True)

    rg = [list(range(N_CORES))]

    with ChunkedDrainTileContext(nc) as tc:
        with (
            tc.tile_pool(name="acts", bufs=1) as acts,
            tc.tile_pool(name="wsm", bufs=22) as wsm,
            tc.tile_pool(name="wdp", bufs=6) as wdp,
            tc.tile_pool(name="imp", bufs=3) as imp,
            tc.tile_pool(name="stat", bufs=2) as stat,
            tc.tile_pool(name="ps", bufs=4, space="PSUM") as psp,
            tc.tile_pool(name="dram", bufs=2, space="DRAM") as dram,
        ):
            def abuf(name, free, dtype=BF):
                return acts.tile([128, free], dtype, tag=name, name=name)

            b1a = abuf("b1a", 98 * 130)
            b1b = abuf("b1b", 98 * 130)
            b2a = abuf("b2a", 98 * 66)
            b2b = abuf("b2b", 98 * 66)
            b3a = abuf("b3a", 98 * 34)
            b3b = abuf("b3b", 98 * 34)
            raw3 = abuf("raw3", 3072)
            b4a = abuf("b4a", 2 * 50 * 18)
            b4b = abuf("b4b", 2 * 50 * 18)
            raw4 = acts.tile([128, 2 * 768], BF, tag="raw3", name="raw4")
            b5a = abuf("b5a", 4 * 26 * 10)
            b5b = abuf("b5b", 4 * 26 * 10)
            raw5 = acts.tile([128, 4 * 192], BF, tag="b3b", name="raw5")
            # stage-6 tiles reuse slots of buffers that are dead by then
            cp6i = acts.tile([128, 4 * 8 * 192], BF, tag="b1a", name="cp6i")
            b6in = abuf("b6in", 4 * 8 * 26 * 10)
            cp6y = acts.tile([128, 8 * 8 * 48], BF, tag="b1b", name="cp6y")
            b6y = acts.tile([128, 8 * 8 * 14 * 6], BF, tag="b2a", name="b6y")
            b6z = acts.tile([128, 8 * 8 * 14 * 6], BF, tag="b2b", name="b6z")
            raw6 = abuf("raw6", 384)
            cmp6 = abuf("cmp6", 384)
            t17 = acts.tile([128, 384], F32, tag="b3a", name="t17")
            cmp5 = acts.tile([128, 4 * 192], BF, tag="raw3", name="cmp5")
            pooled = abuf("pooled", 8, F32)
            bng = abuf("bng", 23, F32)
            bnb = abuf("bnb", 23, F32)
            fcb = abuf("fcb", 1, F32)
            cmagic = abuf("cmagic", 4, U32)
            c15f = abuf("c15f", 4, F32)

            nc.sync.dma_start(bng[:], p_bng[:])
            nc.sync.dma_start(bnb[:], p_bnb[:])
            nc.sync.dma_start(fcb[:], p_fcb[:])
            nc.vector.memset(cmagic[:], MAGIC)
            nc.vector.memset(c15f[:], 1.5)
            # warmup collective: absorbs first-collective setup latency while
            # conv0-conv2 run (no data dependency)
            wci = dram.tile([128, 2], F32, tag="wci", name="wci")
            wco = dram.tile([N_CORES * 128, 2], F32, tag="wco", name="wco")
            nc.sync.dma_start(wci[:], c15f[:, 0:2])
            nc.gpsimd.collective_compute(
                "AllGather", ALU.bypass, replica_groups=rg,
                ins=[wci.opt()], outs=[wco.opt()])

            def view(buf, *dims):
                pat = " ".join(f"d{i}" for i in range(len(dims)))
                kw = {f"d{i}": d for i, d in enumerate(dims)}
                return buf[:].rearrange(f"p ({pat}) -> p {pat}", **kw)

            v1a, v1b = view(b1a, 98, 130), view(b1b, 98, 130)
            v2a, v2b = view(b2a, 98, 66), view(b2b, 98, 66)
            v3a, v3b = view(b3a, 98, 34), view(b3b, 98, 34)
            v4a, v4b = view(b4a, 2, 50, 18), view(b4b, 2, 50, 18)
            v5a, v5b = view(b5a, 4, 26, 10), view(b5b, 4, 26, 10)
            v6in = view(b6in, 4, 8, 26, 10)
            v6y, v6z = view(b6y, 8, 8, 14, 6), view(b6z, 8, 8, 14, 6)

            for v in (v1a, v1b, v2a, v2b, v3a, v3b):
                nc.vector.memset(v[:, :, 0], 0.0)
                nc.vector.memset(v[:, :, v.shape[2] - 1], 0.0)
                nc.vector.memset(v[:, 0, :], 0.0)
                nc.vector.memset(v[:, v.shape[1] - 1, :], 0.0)
            for v in (v4a, v4b, v5a, v5b):
                nc.vector.memset(v[:, :, :, 0], 0.0)
                nc.vector.memset(v[:, :, :, v.shape[3] - 1], 0.0)
                nc.vector.memset(v[:, :, 0, :], 0.0)
                nc.vector.memset(v[:, :, v.shape[2] - 1, :], 0.0)
            for v in (v6in, v6y, v6z):
                nc.vector.memset(v[:, :, :, :, 0], 0.0)
                nc.vector.memset(v[:, :, :, :, v.shape[4] - 1], 0.0)
                nc.vector.memset(v[:, :, :, 0, :], 0.0)
                nc.vector.memset(v[:, :, :, v.shape[3] - 1, :], 0.0)

            def load_w_taps(i, cols):
                ts = []
                for t in range(9):
                    w = wsm.tile([128, cols], BF, tag="w", name=f"w{i}_{t}")
                    nc.sync.dma_start(w[:], p_ws[i][t])
                    ts.append(w)
                return ts

            def halos(buf, bw, pc, rows):
                # next blocks' top halo <- prev blocks' last interior row
                nc.sync.dma_start(
                    buf[bw:128, 0:pc],
                    buf[0:128 - bw, rows * pc:(rows + 1) * pc])
                # prev blocks' bottom halo <- next blocks' first interior row
                nc.sync.dma_start(
                    buf[0:128 - bw, (rows + 1) * pc:(rows + 2) * pc],
                    buf[bw:128, pc:2 * pc])

            # =========== conv0: host im2col, K=9 blockdiag ===========
            w0t = wsm.tile([128, 128], BF, tag="w", name="w0t")
            nc.sync.dma_start(w0t[:], p_ws[0][:])
            for it in range(12):
                imt = imp.tile([128, 1024], BF, name="imt")
                nc.sync.dma_start(imt[:], p_im[:, 1024 * it:1024 * (it + 1)])
                ps = psp.tile([128, 1024], F32, tag="ps", name="ps0")
                for g in range(2):
                    nc.tensor.matmul(ps[:, 512 * g:512 * g + 512], w0t[:],
                                     imt[:, 512 * g:512 * g + 512],
                                     start=True, stop=True)
                nc.scalar.activation(
                    v1a[:, 1 + 8 * it:9 + 8 * it, 1:129], ps[:], AF.Silu)
            halos(b1a, 32, 130, 96)

            # ====== conv1 / conv2: 32ch stride-1, 4-block blockdiag ======
            def conv_s1(src_v, dst_v, dst_buf, wt):
                for it in range(12):
                    ps = psp.tile([128, 1024], F32, tag="ps", name="pss1")
                    for g in range(2):
                        r0 = 8 * it + 4 * g
                        for t in range(9):
                            dy, dx = t // 3, t % 3
                            rhs = src_v[:, r0 + dy:r0 + dy + 4, dx:dx + 128]
                            nc.tensor.matmul(
                                ps[:, 512 * g:512 * g + 512], wt[t][:], rhs,
                                start=(t == 0), stop=(t == 8))
                    nc.scalar.activation(
                        dst_v[:, 1 + 8 * it:9 + 8 * it, 1:129], ps[:], AF.Silu)
                halos(dst_buf, 32, 130, 96)

            conv_s1(v1a, v1b, b1b, load_w_taps(1, 128))
            conv_s1(v1b, v1a, b1a, load_w_taps(2, 128))

            # ====== conv3: 32->64 stride-2, two b-variant matmuls ======
            w3t = load_w_taps(3, 256)
            for it in range(6):
                ps = psp.tile([128, 1024], F32, tag="ps", name="ps3")
                for b in range(2):
                    r0 = 8 * it
                    for t in range(9):
                        dy, dx = t // 3, t % 3
                        rhs = v1a[:, 2 * r0 + dy:2 * r0 + dy + 16:2,
                                        dx:dx + 128:2]
                        nc.tensor.matmul(
                            ps[:, 512 * b:512 * b + 512],
                            w3t[t][:, 128 * b:128 * b + 128], rhs,
                            start=(t == 0), stop=(t == 8))
                for b in range(2):
                    nc.scalar.activation(
                        v2a[:, 1 + 48 * b + 8 * it:9 + 48 * b + 8 * it, 1:65],
                        ps[:, 512 * b:512 * b + 512], AF.Silu)
            halos(b2a, 64, 66, 96)

            # ====== conv4 / conv5: 64ch stride-1, 2-block blockdiag ======
            def conv_s2(src_v, dst_v, dst_buf, wt):
                for it in range(6):
                    ps = psp.tile([128, 1024], F32, tag="ps", name="pss2")
                    for g in range(2):
                        r0 = 16 * it + 8 * g
                        for t in range(9):
                            dy, dx = t // 3, t % 3
                            rhs = src_v[:, r0 + dy:r0 + dy + 8, dx:dx + 64]
                            nc.tensor.matmul(
                                ps[:, 512 * g:512 * g + 512], wt[t][:], rhs,
                                start=(t == 0), stop=(t == 8))
                    nc.scalar.activation(
                        dst_v[:, 1 + 16 * it:17 + 16 * it, 1:65], ps[:], AF.Silu)
                halos(dst_buf, 64, 66, 96)

            conv_s2(v2a, v2b, b2b, load_w_taps(4, 128))
            conv_s2(v2b, v2a, b2a, load_w_taps(5, 128))

            # ---- split-half BN machinery ----
            def bn_stats_part(raw, ranges, mc, nm):
                """ranges: list per m of (start, length) in raw. -> (st2, AG out)"""
                st2 = stat.tile([128, 2 * mc], F32, tag="st2", name="st2" + nm)
                for m, (st, ln) in enumerate(ranges):
                    gdim = (ln + 511) // 512
                    fsz = ln // gdim
                    assert ln % gdim == 0
                    bns = stat.tile([128, 6 * gdim], F32, tag="bns",
                                    name="bns" + nm)
                    for gi in range(gdim):
                        nc.vector.bn_stats(
                            bns[:, 6 * gi:6 * gi + 6],
                            raw[:, st + gi * fsz:st + (gi + 1) * fsz])
                    nc.vector.bn_aggr(st2[:, 2 * m:2 * m + 2],
                                      bns[:].rearrange("p (g f) -> p g f", f=6))
                ccin = dram.tile([128, 2 * mc], F32, tag="ccin", name="cci" + nm)
                ccout = dram.tile([N_CORES * 128, 2 * mc], F32, tag="ccout",
                                  name="cco" + nm)
                nc.sync.dma_start(ccin[:], st2[:])
                nc.gpsimd.collective_compute(
                    "AllGather", ALU.bypass, replica_groups=rg,
                    ins=[ccin.opt()], outs=[ccout.opt()])
                gath = stat.tile([128, 8 * 2 * mc], F32, tag="gath",
                                 name="gth" + nm)
                nc.sync.dma_start(
                    gath[:], ccout[:].rearrange("(r p) m -> p r m", p=128))
                return gath

            def bn_apply(gaths_w, raw, px, mc, slot, dst_views):
                """gaths_w: list of (gath tile, weight). Combine E[x], E[x^2]
                across ranks and halves, then normalize+SiLU."""
                mg = stat.tile([128, mc], F32, tag="mg", name="mg")
                vg = stat.tile([128, mc], F32, tag="vg", name="vg")
                sc1 = stat.tile([128, mc], F32, tag="sc1", name="sc1")
                sc2 = stat.tile([128, mc], F32, tag="sc2", name="sc2")
                t2 = stat.tile([128, mc], F32, tag="t2", name="t2")
                ab = stat.tile([128, 2 * mc], F32, tag="ab", name="ab")
                a, bvec = ab[:, 0:mc], ab[:, mc:2 * mc]
                tmp = stat.tile([128, mc * 8], F32, tag="tmp", name="tmp")
                tv = tmp[:].rearrange("p (m s) -> p m s", m=mc)
                first = True
                for gath, wgt in gaths_w:
                    gv = gath[:].rearrange("p (s m) -> p s m", s=8)
                    means = gv[:, :, 0:2 * mc:2].rearrange("p s m -> p m s")
                    varis = gv[:, :, 1:2 * mc:2].rearrange("p s m -> p m s")
                    nc.vector.reduce_sum(sc1[:], means, axis=mybir.AxisListType.X)
                    nc.vector.tensor_mul(tv, means, means)
                    nc.vector.tensor_add(tv, tv, varis)
                    nc.vector.reduce_sum(sc2[:], tv, axis=mybir.AxisListType.X)
                    if first:
                        nc.vector.tensor_scalar_mul(mg[:], sc1[:], wgt * 0.125)
                        nc.vector.tensor_scalar_mul(vg[:], sc2[:], wgt * 0.125)
                        first = False
                    else:
                        nc.vector.tensor_scalar(sc1[:], sc1[:], wgt * 0.125,
                                                None, op0=ALU.mult)
                        nc.vector.tensor_add(mg[:], mg[:], sc1[:])
                        nc.vector.tensor_scalar(sc2[:], sc2[:], wgt * 0.125,
                                                None, op0=ALU.mult)
                        nc.vector.tensor_add(vg[:], vg[:], sc2[:])
                nc.vector.tensor_mul(sc1[:], mg[:], mg[:])
                nc.vector.tensor_sub(vg[:], vg[:], sc1[:])
                nc.vector.tensor_scalar_add(vg[:], vg[:], EPS)
                vgi = vg[:].bitcast(U32)
                yi = sc1[:].bitcast(U32)
                nc.vector.tensor_scalar(yi, vgi, 1, None,
                                        op0=ALU.logical_shift_right)
                nc.vector.tensor_sub(yi, cmagic[:, 0:mc], yi)
                y = sc1[:]
                h = sc2[:]
                nc.vector.tensor_scalar_mul(h, vg[:], 0.5)
                for _ in range(2):
                    nc.vector.tensor_mul(t2[:], y, y)
                    nc.vector.tensor_mul(t2[:], t2[:], h)
                    nc.vector.tensor_sub(t2[:], c15f[:, 0:mc], t2[:])
                    nc.vector.tensor_mul(y, y, t2[:])
                col = SLOT_COL[slot]
                nc.vector.tensor_mul(a, y, bng[:, col:col + mc])
                nc.vector.tensor_mul(mg[:], mg[:], a)
                nc.vector.tensor_sub(bvec, bnb[:, col:col + mc], mg[:])
                for m in range(mc):
                    nc.scalar.activation(
                        dst_views[m], raw[:, m * px:(m + 1) * px], AF.Silu,
                        bias=ab[:, mc + m:mc + m + 1], scale=ab[:, m:m + 1])

            # ---- BN stats + AllGather + normalize+SiLU (batch-DP) ----
            def bn_silu(raw, px, mc, slot, dst_views):
                g = bn_stats_part(raw, [(m * px, px) for m in range(mc)],
                                  mc, f"f{slot}")
                bn_apply([(g, 1.0)], raw, px, mc, slot, dst_views)

            def bn_silu_local(raw, slot, dst_view):
                bns = stat.tile([128, 6], F32, tag="bns", name="bnsl")
                nc.vector.bn_stats(bns[:], raw[:])
                st2 = stat.tile([128, 2], F32, tag="st2", name="st2l")
                nc.vector.bn_aggr(st2[:], bns[:].rearrange("p (g f) -> p g f", f=6))
                ab = stat.tile([128, 2], F32, tag="ab", name="abl")
                vg = stat.tile([128, 1], F32, tag="vg", name="vgl")
                sc2 = stat.tile([128, 1], F32, tag="sc2", name="sc2l")
                t2 = stat.tile([128, 1], F32, tag="t2", name="t2l")
                nc.vector.tensor_scalar_add(vg[:], st2[:, 1:2], EPS)
                y = ab[:, 0:1]
                yi = y.bitcast(U32)
                nc.vector.tensor_scalar(yi, vg[:].bitcast(U32), 1, None,
                                        op0=ALU.logical_shift_right)
                nc.vector.tensor_sub(yi, cmagic[:, 0:1], yi)
                nc.vector.tensor_scalar_mul(sc2[:], vg[:], 0.5)
                for _ in range(2):
                    nc.vector.tensor_mul(t2[:], y, y)
                    nc.vector.tensor_mul(t2[:], t2[:], sc2[:])
                    nc.vector.tensor_sub(t2[:], c15f[:, 0:1], t2[:])
                    nc.vector.tensor_mul(y, y, t2[:])
                col = SLOT_COL[slot]
                nc.vector.tensor_mul(y, y, bng[:, col:col + 1])
                nc.vector.tensor_mul(vg[:], st2[:, 0:1], y)
                nc.vector.tensor_sub(ab[:, 1:2], bnb[:, col:col + 1], vg[:])
                nc.scalar.activation(dst_view, raw[:], AF.Silu,
                                     bias=ab[:, 1:2], scale=ab[:, 0:1])

            # ====== conv6: 64->128 stride-2, two b-variants -> raw3 ======
            w6t = load_w_taps(6, 256)
            for it in range(3):
                ps = psp.tile([128, 1024], F32, tag="ps", name="ps6")
                for b in range(2):
                    r0 = 16 * it
                    for t in range(9):
                        dy, dx = t // 3, t % 3
                        rhs = v2a[:, 2 * r0 + dy:2 * r0 + dy + 32:2,
                                  dx:dx + 64:2]
                        nc.tensor.matmul(
                            ps[:, 512 * b:512 * b + 512],
                            w6t[t][:, 128 * b:128 * b + 128], rhs,
                            start=(t == 0), stop=(t == 8))
                for b in range(2):
                    nc.vector.tensor_copy(
                        raw3[:, (48 * b + 16 * it) * 32:(48 * b + 16 * it + 16) * 32],
                        ps[:, 512 * b:512 * b + 512])
            bn_silu(raw3[:], 3072, 1, 0, [v3a[:, 1:97, 1:33]])

            # per-tap deep weight tile: [128, kc*mc*128], one DMA per tap
            def wtap(i, t):
                w = wdp.tile([128, KC[i] * MC[i] * 128], BF, tag="wd",
                             name=f"w{i}_{t}")
                nc.sync.dma_start(w[:], p_ws[i][t])
                return w

            # ====== conv7 / conv8: 128ch ======
            def conv_s3(i, src_v, slot, dst_views):
                wt = []
                for t in range(9):
                    w = wsm.tile([128, 128], BF, tag="w", name=f"w{i}_{t}")
                    nc.sync.dma_start(w[:], p_ws[i][t])
                    wt.append(w)
                for g in range(3):
                    ps = psp.tile([128, 1024], F32, tag="ps", bufs=4,
                                  name=f"ps{i}_{g}")
                    for t in range(9):
                        dy, dx = t // 3, t % 3
                        for h in range(2):
                            r0 = 32 * g + 16 * h
                            rhs = src_v[:, r0 + dy:r0 + dy + 16, dx:dx + 32]
                            nc.tensor.matmul(
                                ps[:, 512 * h:512 * h + 512], wt[t][:], rhs,
                                start=(t == 0), stop=(t == 8))
                    nc.vector.tensor_copy(
                        raw3[:, 1024 * g:1024 * (g + 1)], ps[:])
                bn_silu(raw3[:], 3072, 1, slot, dst_views)

            conv_s3(7, v3a, 1, [v3b[:, 1:97, 1:33]])
            conv_s3(8, v3b, 2, [v3a[:, 1:97, 1:33]])

            # ====== conv9: 128->256 stride-2 ======
            wt9 = []
            for t in range(9):
                w = wsm.tile([128, 256], BF, tag="w", name=f"w9_{t}")
                nc.sync.dma_start(w[:], p_ws[9][t])
                wt9.append(w)
            for cch in range(2):
                for m in range(2):
                    ps = psp.tile([128, 384], F32, tag="ps", bufs=4,
                                  name=f"ps9_{m}_{cch}")
                    r0 = 24 * cch
                    for t in range(9):
                        dy, dx = t // 3, t % 3
                        rhs = v3a[:, 2 * r0 + dy:2 * r0 + dy + 48:2,
                                  dx:dx + 32:2]
                        nc.tensor.matmul(
                            ps[:], wt9[t][:, 128 * m:128 * m + 128],
                            rhs, start=(t == 0), stop=(t == 8))
                    nc.vector.tensor_copy(
                        raw4[:, m * 768 + 384 * cch:m * 768 + 384 * (cch + 1)],
                        ps[:])
            bn_silu(raw4[:], 768, 2, 3, [v4a[:, m, 1:49, 1:17] for m in range(2)])

            # ====== conv10 / conv11: 256ch ======
            def conv_s4(i, src_v, slot, dst_views):
                wt = []
                for t in range(9):
                    w = wsm.tile([128, 512], BF, tag="w", name=f"w{i}_{t}")
                    nc.sync.dma_start(w[:], p_ws[i][t])
                    wt.append(w)
                for cch in range(2):
                    for m in range(2):
                        ps = psp.tile([128, 384], F32, tag="ps", bufs=4,
                                      name=f"ps{i}_{m}_{cch}")
                        r0 = 24 * cch
                        acc = 0
                        for t in range(9):
                            dy, dx = t // 3, t % 3
                            for k in range(2):
                                acc += 1
                                rhs = src_v[:, k, r0 + dy:r0 + dy + 24,
                                            dx:dx + 16]
                                nc.tensor.matmul(
                                    ps[:],
                                    wt[t][:, (2 * k + m) * 128:(2 * k + m) * 128 + 128],
                                    rhs, start=(acc == 1), stop=(acc == 18))
                        nc.vector.tensor_copy(
                            raw4[:, m * 768 + 384 * cch:m * 768 + 384 * (cch + 1)],
                            ps[:])
                bn_silu(raw4[:], 768, 2, slot, dst_views)

            conv_s4(10, v4a, 4, [v4b[:, m, 1:49, 1:17] for m in range(2)])
            conv_s4(11, v4b, 5, [v4a[:, m, 1:49, 1:17] for m in range(2)])

            # ====== conv12: 256->512 stride-2 ======
            pss = [psp.tile([128, 192], F32, tag="ps", bufs=4,
                            name=f"ps12_{j}") for j in range(4)]
            acc = 0
            for t in range(9):
                dy, dx = t // 3, t % 3
                w = wtap(12, t)
                for k in range(2):
                    acc += 1
                    for m in range(4):
                        rhs = v4a[:, k, dy:dy + 48:2, dx:dx + 16:2]
                        nc.tensor.matmul(
                            pss[m][:],
                            w[:, (4 * k + m) * 128:(4 * k + m) * 128 + 128],
                            rhs, start=(acc == 1), stop=(acc == 18))
            for m in range(4):
                nc.vector.tensor_copy(raw5[:, m * 192:(m + 1) * 192], pss[m][:])
            bn_silu(raw5[:], 192, 4, 6, [v5a[:, m, 1:25, 1:9] for m in range(4)])

            # ====== conv13 / conv14: 512ch ======
            def conv_s5(i, src_v, slot, dst_views):
                pss = [psp.tile([128, 192], F32, tag="ps", bufs=4,
                                name=f"ps{i}_{j}") for j in range(4)]
                acc = 0
                for t in range(9):
                    dy, dx = t // 3, t % 3
                    w = wtap(i, t)
                    for k in range(4):
                        acc += 1
                        for m in range(4):
                            rhs = src_v[:, k, dy:dy + 24, dx:dx + 8]
                            nc.tensor.matmul(
                                pss[m][:],
                                w[:, (4 * k + m) * 128:(4 * k + m) * 128 + 128],
                                rhs, start=(acc == 1), stop=(acc == 36))
                for m in range(4):
                    nc.vector.tensor_copy(raw5[:, m * 192:(m + 1) * 192],
                                          pss[m][:])
                bn_silu(raw5[:], 192, 4, slot, dst_views)

            conv_s5(13, v5a, 7, [v5b[:, m, 1:25, 1:9] for m in range(4)])
            conv_s5(14, v5b, 8, [cmp5[:, m * 192:(m + 1) * 192] for m in range(4)])

            # ====== stage 6: channel-parallel, AllGather activations ======
            cc6i = dram.tile([128, 4 * 192], BF, tag="cc6i", name="cc6i")
            cc6o = dram.tile([N_CORES * 128, 4 * 192], BF, tag="cc6o",
                             name="cc6o")
            nc.sync.dma_start(cc6i[:], cmp5[:])
            nc.gpsimd.collective_compute(
                "AllGather", ALU.bypass, replica_groups=rg,
                ins=[cc6i.opt()], outs=[cc6o.opt()])
            cc6v = cc6o[:].rearrange("(r p) px -> p r px", p=128)
            nc.sync.dma_start(cp6i[:].rearrange("p (r px) -> p r px", r=8),
                              cc6v)
            # scatter compact [p, s, k, a, b] -> padded (DVE, strided APs)
            cp6iv = cp6i[:].rearrange("p (s k a b) -> p s k a b",
                                      s=8, k=4, a=24)
            for k in range(4):
                nc.vector.tensor_copy(
                    v6in[:, k, :, 1:25, 1:9],
                    cp6iv[:, :, k].rearrange("p s a b -> p s a b"))

            def stage6_conv(i, rhs_of, dst_raw, copy_on_dve=True):
                ps = psp.tile([128, 384], F32, tag="ps", bufs=4,
                              name=f"ps{i}")
                acc = 0
                nk = KC[i]
                for t in range(9):
                    dy, dx = t // 3, t % 3
                    w = wtap(i, t)
                    for k in range(nk):
                        acc += 1
                        nc.tensor.matmul(
                            ps[:], w[:, 128 * k:128 * k + 128],
                            rhs_of(k, dy, dx),
                            start=(acc == 1), stop=(acc == 9 * nk))
                nc.vector.tensor_copy(dst_raw[:], ps[:])

            def ag_to(dst_v, src_cmp, tag):
                ci = dram.tile([128, 384], BF, tag=tag + "i", name=tag + "i")
                co = dram.tile([N_CORES * 128, 384], BF, tag=tag + "o",
                               name=tag + "o")
                nc.sync.dma_start(ci[:], src_cmp[:])
                nc.gpsimd.collective_compute(
                    "AllGather", ALU.bypass, replica_groups=rg,
                    ins=[ci.opt()], outs=[co.opt()])
                cov = co[:].rearrange("(k p) px -> p k px", p=128)
                nc.sync.dma_start(
                    cp6y[:].rearrange("p (k px) -> p k px", k=8), cov)
                cpv = cp6y[:].rearrange("p (k s a b) -> p k s a b",
                                        k=8, s=8, a=12)
                for k in range(8):
                    nc.vector.tensor_copy(dst_v[:, k, :, 1:13, 1:5], cpv[:, k])

            stage6_conv(15, lambda k, dy, dx:
                        v6in[:, k, :, dy:dy + 24:2, dx:dx + 8:2], raw6)
            bn_silu_local(raw6, 9,
                          cmp6[:].rearrange("p (s a b) -> p s a b", s=8, a=12))
            ag_to(v6y, cmp6, "ag15")

            stage6_conv(16, lambda k, dy, dx:
                        v6y[:, k, :, dy:dy + 12, dx:dx + 4], raw6)
            bn_silu_local(raw6, 10,
                          cmp6[:].rearrange("p (s a b) -> p s a b", s=8, a=12))
            ag_to(v6z, cmp6, "ag16")

            stage6_conv(17, lambda k, dy, dx:
                        v6z[:, k, :, dy:dy + 12, dx:dx + 4], t17)
            nc.vector.reduce_sum(
                pooled[:], t17[:].rearrange("p (s f) -> p s f", s=8),
                axis=mybir.AxisListType.X)

            # ====== FC (fp32 matmul, all samples, redundant per core) ======
            fci = dram.tile([128, 8], F32, tag="fci", name="fci")
            fco = dram.tile([N_CORES * 128, 8], F32, tag="fco", name="fco")
            nc.sync.dma_start(fci[:], pooled[:])
            nc.gpsimd.collective_compute(
                "AllGather", ALU.bypass, replica_groups=rg,
                ins=[fci.opt()], outs=[fco.opt()])
            fcov = fco[:].rearrange("(k p) s -> p k s", p=128)
            psf = psp.tile([128, 8], F32, tag="ps", bufs=4, name="psf")
            frs = stat.tile([128, 64], F32, tag="fr", name="frs")
            nc.sync.dma_start(frs[:].rearrange("p (k s) -> p k s", k=8), fcov)
            for k in range(8):
                fw = wdp.tile([128, 128], F32, tag="wd", name=f"fw{k}")
                nc.sync.dma_start(fw[:], p_fcw[k])
                nc.tensor.matmul(psf[:], fw[:], frs[:, 8 * k:8 * k + 8],
                                 start=(k == 0), stop=(k == 7))
            fout = stat.tile([128, 8], F32, tag="fout", name="fout")
            nc.vector.tensor_scalar_add(fout[:], psf[:], fcb[:, 0:1])
            nc.sync.dma_start(p_out[:], fout[:])
    return nc


# ====================== host side ======================

_CACHED_NC = None


def _get_nc():
    global _CACHED_NC
    if _CACHED_NC is None:
        _CACHED_NC = build_nc()
    return _CACHED_NC


def _to_bf(a):
    return np.asarray(a, np.float32).astype(ml_dtypes.bfloat16)


def _im2col0(xs):
    """xs: [768, 256]. -> [128, 12288] bf16: partitions 32q+t hold tap t of
    quarter q (stride-2, pad 1), 96 rows x 128 cols each."""
    xp = np.zeros((770, 258), np.float32)
    xp[1:769, 1:257] = xs
    out = np.zeros((128, 12288), np.float32)
    for q in range(4):
        for t in range(9):
            dy, dx = t // 3, t % 3
            rows = xp[192 * q + dy:192 * q + dy + 192:2, dx:dx + 256:2]
            out[32 * q + t] = rows.reshape(-1)
    return _to_bf(out)


def _pack_weights(conv_ws):
    w = [np.asarray(x, np.float32) for x in conv_ws]
    d = {}
    # conv0 blockdiag [128, 128]: rows 32q+t -> cols 32q..32q+31
    a = np.zeros((128, 128), np.float32)
    for q in range(4):
        for t in range(9):
            a[32 * q + t, 32 * q:32 * q + 32] = w[0][t // 3, t % 3, 0, :]
    d["w0"] = _to_bf(a)
    # conv1/2: [9, 128, 128] 4-block diagonal
    for i in (1, 2):
        a = np.zeros((9, 128, 128), np.float32)
        for t in range(9):
            blk = w[i][t // 3, t % 3]          # [32, 32]
            for q in range(4):
                a[t, 32 * q:32 * q + 32, 32 * q:32 * q + 32] = blk
        d[f"w{i}"] = _to_bf(a)
    # conv3: [9, 128, 256]; variant b at cols 128b: rows 32(2h+b) -> 64h
    a = np.zeros((9, 128, 256), np.float32)
    for t in range(9):
        blk = w[3][t // 3, t % 3]              # [32, 64]
        for b in range(2):
            for h in range(2):
                qb = 2 * h + b
                a[t, 32 * qb:32 * qb + 32, 128 * b + 64 * h:128 * b + 64 * h + 64] = blk
    d["w3"] = _to_bf(a)
    # conv4/5: [9, 128, 128] 2-block diagonal of [64, 64]
    for i in (4, 5):
        a = np.zeros((9, 128, 128), np.float32)
        for t in range(9):
            blk = w[i][t // 3, t % 3]          # [64, 64]
            for h in range(2):
                a[t, 64 * h:64 * h + 64, 64 * h:64 * h + 64] = blk
        d[f"w{i}"] = _to_bf(a)
    # conv6: [9, 128, 256]; variant b at cols 128b: rows 64b -> all 128
    a = np.zeros((9, 128, 256), np.float32)
    for t in range(9):
        blk = w[6][t // 3, t % 3]              # [64, 128]
        for b in range(2):
            a[t, 64 * b:64 * b + 64, 128 * b:128 * b + 128] = blk
    d["w6"] = _to_bf(a)
    # conv7..14: [9, 128, kc*mc*128]; free index = (k*mc + m)*128 + co
    for i in range(7, 15):
        cin, cout, _ = CONV_SPECS[i]
        kc, mc = KC[i], MC[i]
        a = np.zeros((9, 128, kc * mc * 128), np.float32)
        for t in range(9):
            for k in range(kc):
                a[t, :, k * mc * 128:(k + 1) * mc * 128] = \
                    w[i][t // 3, t % 3, 128 * k:128 * k + 128, :]
        d[f"w{i}"] = _to_bf(a)
    # conv15..17: per-core cout slice; [9, 128, kc*128]
    for i in range(15, 18):
        kc = KC[i]
        percore = []
        for c in range(N_CORES):
            a = np.zeros((9, 128, kc * 128), np.float32)
            for t in range(9):
                for k in range(kc):
                    a[t, :, 128 * k:128 * k + 128] = \
                        w[i][t // 3, t % 3, 128 * k:128 * k + 128,
                             128 * c:128 * c + 128]
            percore.append(_to_bf(a))
        d[f"w{i}"] = percore
    return d


def _pack_bn(bn_gammas, bn_betas):
    gs = [np.asarray(g, np.float32) for g in bn_gammas]
    bs = [np.asarray(b, np.float32) for b in bn_betas]
    outg = [np.zeros((128, 23), np.float32) for _ in range(N_CORES)]
    outb = [np.zeros((128, 23), np.float32) for _ in range(N_CORES)]
    for slot in range(11):
        col, mc = SLOT_COL[slot], BN_MC[slot]
        for c in range(N_CORES):
            for m in range(mc):
                if slot >= 9:
                    sl = slice(128 * c, 128 * c + 128)
                else:
                    sl = slice(128 * m, 128 * m + 128)
                outg[c][:, col + m] = gs[slot][sl]
                outb[c][:, col + m] = bs[slot][sl]
    return outg, outb


def _run(inputs, trace=False):
    x = np.asarray(inputs["x"], np.float32)
    wd = _pack_weights(inputs["conv_ws"])
    bg, bb = _pack_bn(inputs["bn_gammas"], inputs["bn_betas"])
    fc_w = np.asarray(inputs["fc_w"], np.float32)
    fc_b = np.asarray(inputs["fc_b"], np.float32)
    fcw = np.zeros((8, 128, 128), np.float32)
    for k in range(8):
        fcw[k] = fc_w[:, 128 * k:128 * k + 128].T / 48.0
    fcb = fc_b.reshape(128, 1).astype(np.float32)

    in_maps = []
    for c in range(N_CORES):
        m = {"im2col": _im2col0(x[c, 0]), "fcw": fcw, "fcb": fcb,
             "bng": bg[c], "bnb": bb[c]}
        for i in range(18):
            key = f"w{i}"
            m[key] = wd[key][c] if i >= 15 else wd[key]
        in_maps.append(m)

    nc = _get_nc()
    res = run_bass_kernel_spmd(nc, in_maps, list(range(N_CORES)), trace=trace)
    out = np.ascontiguousarray(res.results[0]["fc"].T.astype(np.float32))
    return out, res


def kernel(**inputs):
    out, _ = _run(inputs, trace=False)
    return out
